# revision 1
# baseline (speedup 1.0000x reference)
"""AtomPlacementScheduler Trainium2 kernel.

out[b] = sum_e irfft(rfft(stems[b,e]) * exp(-2i pi f s_be)),  s = sigmoid(TL@W+b)*N.

Implemented as a 4-step FFT (N = 32768 = 128 x 256) so all heavy work is
TensorEngine matmuls; the per-event shift phase factors into A[k2] (folded into
the stage-2 twiddle multiply) and B[k1] (folded into the per-event stage-3 DFT
matrix), and the sum over 16 events is free PSUM accumulation.  Pure data
parallel over batch: 64 batches / 8 cores = 8 per core.

Self-contained: hardcodes shapes B=64, E=16, N=32768, n_cores=8.
"""
import numpy as np
import ml_dtypes

N = 32768
N1 = 128   # stage-3 DFT size
N2 = 256   # stage-1 DFT size
E = 16
B = 64
NCORES = 8
BC = B // NCORES      # 8 batches per core
S = BC * E            # 128 signals per core
K1 = 65               # k1 = 0..64 covers k = k2 + 256*k1 up to Nyquist

F32 = np.float32
BF16 = ml_dtypes.bfloat16


def _host_tables():
    n1 = np.arange(N1)
    n2 = np.arange(N2)
    k2 = np.arange(N2)
    k1 = np.arange(K1)
    W2 = np.exp(-2j * np.pi * np.outer(n2, k2) / N2)        # (n2, k2)
    W2cat = np.concatenate([W2.real, W2.imag], 1)           # (256, 512)
    T = np.exp(-2j * np.pi * np.outer(n1, k2) / N)          # (n1, k2)
    W1 = np.exp(-2j * np.pi * np.outer(n1, k1) / N1)        # (n1, k1)
    E1 = np.exp(+2j * np.pi * np.outer(np.arange(64), np.arange(N1)) / N1)  # (k1, m)
    Tinv = np.exp(+2j * np.pi * np.outer(np.arange(N1), k2) / N)            # (m, k2)
    E2 = np.exp(+2j * np.pi * np.outer(k2, np.arange(N2)) / N2) * (2.0 / N)  # (k2, j)
    return W2cat, T, W1, E1, Tinv, E2


def _build_graph():
    import concourse.bass as bass
    import concourse.mybir as mybir
    import concourse.tile as tile
    from concourse import bacc

    dt = mybir.dt
    nc = bacc.Bacc("TRN2", target_bir_lowering=False, debug=False, num_devices=NCORES)

    # ---- DRAM parameters (per-core shard shapes) ----
    stems_d = nc.dram_tensor("stems", [BC, E, N2, N1], dt.float32, kind="ExternalInput")
    a_d = nc.dram_tensor("a_tab", [1, S * 512], dt.bfloat16, kind="ExternalInput")
    b_d = nc.dram_tensor("b_tab", [1, S * 130], dt.bfloat16, kind="ExternalInput")
    w2_d = nc.dram_tensor("w2cat", [N2, 512], dt.bfloat16, kind="ExternalInput")
    tc_d = nc.dram_tensor("t_c", [N1, N2], dt.bfloat16, kind="ExternalInput")
    ts_d = nc.dram_tensor("t_s", [N1, N2], dt.bfloat16, kind="ExternalInput")
    w1_d = nc.dram_tensor("w1cs", [N1, 130], dt.bfloat16, kind="ExternalInput")   # [W1c|W1s]
    e1c_d = nc.dram_tensor("e1c", [64, N1], dt.bfloat16, kind="ExternalInput")
    e1s_d = nc.dram_tensor("e1s", [64, N1], dt.bfloat16, kind="ExternalInput")
    tic_d = nc.dram_tensor("ti_c", [N1, N2], dt.bfloat16, kind="ExternalInput")
    tis_d = nc.dram_tensor("ti_s", [N1, N2], dt.bfloat16, kind="ExternalInput")
    e2c_d = nc.dram_tensor("e2c", [N2, N2], dt.bfloat16, kind="ExternalInput")
    e2sn_d = nc.dram_tensor("e2sn", [N2, N2], dt.bfloat16, kind="ExternalInput")
    ones_d = nc.dram_tensor("ones", [1, 128], dt.bfloat16, kind="ExternalInput")
    out_d = nc.dram_tensor("out", [BC, N2, N1], dt.float32, kind="ExternalOutput")
    aux_d = nc.dram_tensor("aux", [BC, 2], dt.float32, kind="ExternalOutput")

    with tile.TileContext(nc) as tc:
        with (
            tc.tile_pool(name="const", bufs=1) as cpool,
            tc.tile_pool(name="work", bufs=3) as pool,
            tc.tile_pool(name="psum", bufs=2, space="PSUM") as psum,
            tc.tile_pool(name="psacc", bufs=1, space="PSUM") as psacc,
        ):
            # ---- load constants once ----
            w2_0 = cpool.tile([128, 512], dt.bfloat16, tag="w2_0")
            w2_1 = cpool.tile([128, 512], dt.bfloat16, tag="w2_1")
            w2 = [w2_0, w2_1]
            nc.sync.dma_start(w2[0][:], w2_d[0:128, :])
            nc.sync.dma_start(w2[1][:], w2_d[128:256, :])
            t_c = cpool.tile([N1, N2], dt.bfloat16, tag="tc")
            t_s = cpool.tile([N1, N2], dt.bfloat16, tag="ts")
            nc.sync.dma_start(t_c[:], tc_d[:])
            nc.sync.dma_start(t_s[:], ts_d[:])
            w1 = cpool.tile([N1, 130], dt.bfloat16, tag="w1")
            nc.sync.dma_start(w1[:], w1_d[:])
            a_sb = cpool.tile([1, S * 512], dt.bfloat16, tag="a")
            nc.sync.dma_start(a_sb[:], a_d[:])
            b_sb = cpool.tile([1, S * 130], dt.bfloat16, tag="b")
            nc.sync.dma_start(b_sb[:], b_d[:])
            e1c = cpool.tile([64, N1], dt.bfloat16, tag="e1c")
            e1s = cpool.tile([64, N1], dt.bfloat16, tag="e1s")
            nc.sync.dma_start(e1c[:], e1c_d[:])
            nc.sync.dma_start(e1s[:], e1s_d[:])
            ti_c = cpool.tile([N1, N2], dt.bfloat16, tag="tic")
            ti_s = cpool.tile([N1, N2], dt.bfloat16, tag="tis")
            nc.sync.dma_start(ti_c[:], tic_d[:])
            nc.sync.dma_start(ti_s[:], tis_d[:])
            e2c_0 = cpool.tile([128, N2], dt.bfloat16, tag="e2c_0")
            e2c_1 = cpool.tile([128, N2], dt.bfloat16, tag="e2c_1")
            e2sn_0 = cpool.tile([128, N2], dt.bfloat16, tag="e2sn_0")
            e2sn_1 = cpool.tile([128, N2], dt.bfloat16, tag="e2sn_1")
            e2c = [e2c_0, e2c_1]
            e2sn = [e2sn_0, e2sn_1]
            nc.sync.dma_start(e2c[0][:], e2c_d[0:128, :])
            nc.sync.dma_start(e2c[1][:], e2c_d[128:256, :])
            nc.sync.dma_start(e2sn[0][:], e2sn_d[0:128, :])
            nc.sync.dma_start(e2sn[1][:], e2sn_d[128:256, :])
            ones = cpool.tile([1, 128], dt.bfloat16, tag="ones")
            nc.sync.dma_start(ones[:], ones_d[:])

            for b in range(BC):
                pA = psacc.tile([K1, 512], dt.float32, tag="pA")
                pB = psacc.tile([K1, 512], dt.float32, tag="pB")
                for e in range(E):
                    sig = b * E + e
                    # stage 1: xm chunks (cast f32->bf16 via gpsimd dma)
                    xm0 = pool.tile([128, N1], dt.bfloat16, tag="xm0")
                    xm1 = pool.tile([128, N1], dt.bfloat16, tag="xm1")
                    nc.gpsimd.dma_start(xm0[:], stems_d[b, e, 0:128, :])
                    nc.gpsimd.dma_start(xm1[:], stems_d[b, e, 128:256, :])
                    p1 = psum.tile([N1, 512], dt.float32, tag="p1")
                    nc.tensor.matmul(p1[:], xm0[:], w2[0][:], start=True, stop=False)
                    nc.tensor.matmul(p1[:], xm1[:], w2[1][:], start=False, stop=True)
                    # broadcast A and B rows across partitions via ones-matmul
                    pab = psum.tile([128, 512], dt.float32, tag="pbc")
                    nc.tensor.matmul(pab[:], ones[:],
                                     a_sb[0:1, sig * 512 : sig * 512 + 512],
                                     start=True, stop=True)
                    pbb = psum.tile([128, 512], dt.float32, tag="pbc")
                    nc.tensor.matmul(pbb[:, 0:130], ones[:],
                                     b_sb[0:1, sig * 130 : sig * 130 + 130],
                                     start=True, stop=True)
                    ab = pool.tile([128, 512], dt.bfloat16, tag="ab")
                    nc.any.tensor_copy(ab[:], pab[:])
                    bb = pool.tile([128, 130], dt.bfloat16, tag="bb")
                    nc.any.tensor_copy(bb[:], pbb[:, 0:130])
                    # C = T * A  (complex), Cc/Cs (128,256) bf16
                    cc = pool.tile([N1, N2], dt.bfloat16, tag="cc")
                    cs = pool.tile([N1, N2], dt.bfloat16, tag="cs")
                    tmp1 = pool.tile([N1, N2], dt.bfloat16, tag="tmp1")
                    tmp2 = pool.tile([N1, N2], dt.bfloat16, tag="tmp2")
                    nc.any.tensor_mul(tmp1[:], t_c[:], ab[:, 0:256])
                    nc.any.tensor_mul(tmp2[:], t_s[:], ab[:, 256:512])
                    nc.any.tensor_sub(cc[:], tmp1[:], tmp2[:])
                    nc.any.tensor_mul(tmp1[:], t_c[:], ab[:, 256:512])
                    nc.any.tensor_mul(tmp2[:], t_s[:], ab[:, 0:256])
                    nc.any.tensor_add(cs[:], tmp1[:], tmp2[:])
                    # UV = inner * C: U = Pre*Cc - Pim*Cs ; V = Pre*Cs + Pim*Cc
                    uv = pool.tile([N1, 512], dt.bfloat16, tag="uv")
                    nc.any.tensor_mul(tmp1[:], p1[:, 0:256], cc[:])
                    nc.any.tensor_mul(tmp2[:], p1[:, 256:512], cs[:])
                    nc.any.tensor_sub(uv[:, 0:256], tmp1[:], tmp2[:])
                    nc.any.tensor_mul(tmp1[:], p1[:, 0:256], cs[:])
                    nc.any.tensor_mul(tmp2[:], p1[:, 256:512], cc[:])
                    nc.any.tensor_add(uv[:, 256:512], tmp1[:], tmp2[:])
                    # M = W1 * B (complex), (128, 65) each
                    m_re = pool.tile([N1, K1], dt.bfloat16, tag="mre")
                    m_im = pool.tile([N1, K1], dt.bfloat16, tag="mim")
                    st1 = pool.tile([N1, K1], dt.bfloat16, tag="st1")
                    st2 = pool.tile([N1, K1], dt.bfloat16, tag="st2")
                    nc.any.tensor_mul(st1[:], w1[:, 0:65], bb[:, 0:65])
                    nc.any.tensor_mul(st2[:], w1[:, 65:130], bb[:, 65:130])
                    nc.any.tensor_sub(m_re[:], st1[:], st2[:])
                    nc.any.tensor_mul(st1[:], w1[:, 0:65], bb[:, 65:130])
                    nc.any.tensor_mul(st2[:], w1[:, 65:130], bb[:, 0:65])
                    nc.any.tensor_add(m_im[:], st1[:], st2[:])
                    # stage 3, accumulating over events
                    nc.tensor.matmul(pA[:], m_re[:], uv[:],
                                     start=(e == 0), stop=(e == E - 1))
                    nc.tensor.matmul(pB[:], m_im[:], uv[:],
                                     start=(e == 0), stop=(e == E - 1))
                # combine into X (65, 512) bf16: [Xre|Xim]
                xf = pool.tile([K1, 512], dt.bfloat16, tag="xf")
                pbsb = pool.tile([K1, 512], dt.bfloat16, tag="pbsb")
                nc.any.tensor_copy(pbsb[:], pB[:])
                nc.any.tensor_sub(xf[:, 0:256], pA[:, 0:256], pbsb[:, 256:512])
                nc.any.tensor_add(xf[:, 256:512], pA[:, 256:512], pbsb[:, 0:256])
                nc.gpsimd.dma_start(aux_d[b, 0:1], xf[0:1, 0:1])
                nc.gpsimd.dma_start(aux_d[b, 1:2], xf[64:65, 0:1])
                # inverse stage I1: G = E1^T @ X   (contract k1=64)
                pga = psum.tile([N1, 512], dt.float32, tag="pinv")
                pgb = psum.tile([N1, 512], dt.float32, tag="pinv")
                nc.tensor.matmul(pga[:], e1c[:], xf[0:64, :], start=True, stop=True)
                nc.tensor.matmul(pgb[:], e1s[:], xf[0:64, :], start=True, stop=True)
                g_re = pool.tile([N1, N2], dt.bfloat16, tag="gre")
                g_im = pool.tile([N1, N2], dt.bfloat16, tag="gim")
                gbsb = pool.tile([N1, 512], dt.bfloat16, tag="gbsb")
                nc.any.tensor_copy(gbsb[:], pgb[:])
                nc.any.tensor_sub(g_re[:], pga[:, 0:256], gbsb[:, 256:512])
                nc.any.tensor_add(g_im[:], pga[:, 256:512], gbsb[:, 0:256])
                # twiddle: GT = G * Tinv
                gt_re = pool.tile([N1, N2], dt.bfloat16, tag="gtre")
                gt_im = pool.tile([N1, N2], dt.bfloat16, tag="gtim")
                it1 = pool.tile([N1, N2], dt.bfloat16, tag="it1")
                it2 = pool.tile([N1, N2], dt.bfloat16, tag="it2")
                nc.any.tensor_mul(it1[:], g_re[:], ti_c[:])
                nc.any.tensor_mul(it2[:], g_im[:], ti_s[:])
                nc.any.tensor_sub(gt_re[:], it1[:], it2[:])
                nc.any.tensor_mul(it1[:], g_re[:], ti_s[:])
                nc.any.tensor_mul(it2[:], g_im[:], ti_c[:])
                nc.any.tensor_add(gt_im[:], it1[:], it2[:])
                # transpose GT -> (k2, m), 2 chunks each
                gtt_re_0 = pool.tile([128, N1], dt.bfloat16, tag="gttre0")
                gtt_re_1 = pool.tile([128, N1], dt.bfloat16, tag="gttre1")
                gtt_im_0 = pool.tile([128, N1], dt.bfloat16, tag="gttim0")
                gtt_im_1 = pool.tile([128, N1], dt.bfloat16, tag="gttim1")
                gtt_re = [gtt_re_0, gtt_re_1]
                gtt_im = [gtt_im_0, gtt_im_1]
                for kc in range(2):
                    nc.sync.dma_start_transpose(
                        gtt_re[kc][:], gt_re[:, 128 * kc : 128 * kc + 128])
                    nc.sync.dma_start_transpose(
                        gtt_im[kc][:], gt_im[:, 128 * kc : 128 * kc + 128])
                # I4: S[j, m] = sum_k2 E2[k2,j] GTt[k2,m]  (real part only)
                for jc in range(2):
                    ps = psum.tile([128, 512], dt.float32, tag="pinv")
                    for kc in range(2):
                        nc.tensor.matmul(
                            ps[:, 0:128], e2c[kc][:, 128 * jc : 128 * jc + 128],
                            gtt_re[kc][:],
                            start=(kc == 0), stop=False)
                        nc.tensor.matmul(
                            ps[:, 0:128], e2sn[kc][:, 128 * jc : 128 * jc + 128],
                            gtt_im[kc][:],
                            start=False, stop=(kc == 1))
                    y_sb = pool.tile([128, N1], dt.float32, tag="ysb")
                    nc.any.tensor_copy(y_sb[:], ps[:, 0:128])
                    nc.sync.dma_start(out_d[b, 128 * jc : 128 * jc + 128, :], y_sb[:])
    nc.compile()
    return nc


def kernel(time_latent, stems, targets, W_pos, b_pos):
    from concourse.bass_utils import run_bass_kernel_spmd

    # host: positions (tiny linear+sigmoid, fp32 exactly like the reference)
    z = np.einsum("bed,od->beo", time_latent.astype(F32), W_pos.astype(F32))
    z = z.reshape(B, E) + b_pos.reshape(1)[0]
    pos = 1.0 / (1.0 + np.exp(-z, dtype=F32))
    s = pos * np.float32(N)

    W2cat, T, W1, E1, Tinv, E2 = _host_tables()
    k2 = np.arange(N2)
    k1 = np.arange(K1)

    nc = _build_graph()
    in_maps = []
    for c in range(NCORES):
        sl = slice(c * BC, (c + 1) * BC)
        s_flat = s[sl].reshape(-1).astype(np.float64)
        A = np.exp(-2j * np.pi * np.outer(s_flat, k2) / N)
        Bt = np.exp(-2j * np.pi * np.outer(s_flat, k1) / N1)
        in_maps.append({
            "stems": np.ascontiguousarray(stems[sl]).reshape(BC, E, N2, N1),
            "a_tab": np.concatenate([A.real, A.imag], 1).astype(BF16).reshape(1, -1),
            "b_tab": np.concatenate([Bt.real, Bt.imag], 1).astype(BF16).reshape(1, -1),
            "w2cat": W2cat.astype(BF16),
            "t_c": T.real.astype(BF16),
            "t_s": T.imag.astype(BF16),
            "w1cs": np.concatenate([W1.real, W1.imag], 1).astype(BF16),
            "e1c": E1.real.astype(BF16),
            "e1s": E1.imag.astype(BF16),
            "ti_c": Tinv.real.astype(BF16),
            "ti_s": Tinv.imag.astype(BF16),
            "e2c": E2.real.astype(BF16),
            "e2sn": (-E2.imag).astype(BF16),
            "ones": np.ones((1, 128), dtype=BF16),
        })

    import os
    trace = bool(int(os.environ.get("ATHENA_TRACE", "0")))
    res = run_bass_kernel_spmd(nc, in_maps, core_ids=list(range(NCORES)), trace=trace)
    if trace:
        print(f"HW exec time: {res.exec_time_ns} ns")
    outs = []
    sign = np.where(np.arange(N) % 2 == 0, 1.0, -1.0).astype(F32)
    for c in range(NCORES):
        y = res.results[c]["out"].reshape(BC, N).astype(F32)
        aux = res.results[c]["aux"].astype(F32)          # (BC, 2) = X0, XN2re
        y = y + (-aux[:, 0:1] + sign[None, :] * aux[:, 1:2]) / np.float32(N)
        outs.append(y)
    return np.concatenate(outs, 0).reshape(B, 1, N).astype(F32)



# revision 3
# speedup vs baseline: 1.3612x; 1.3612x over previous
"""AtomPlacementScheduler Trainium2 kernel (v2).

out[b] = sum_e irfft(rfft(stems[b,e]) * exp(-2i pi f s_be)),  s = sigmoid(TL@W+b)*N.

4-step FFT (N = 32768 = 128 x 256).  v2: all per-event twiddle tables
(C = T*A_e, M = W1*B_e) are precomputed on host and DMA'd in fp16, so the
device does only: 2 stage-1 matmuls, 1 PSUM->SBUF copy, 6 elementwise ops
(complex multiply by C), 2 stage-3 matmuls per event.  Event sum is free PSUM
accumulation.  Pure data parallel over batch: 64 batches / 8 cores.

Self-contained: hardcodes shapes B=64, E=16, N=32768, n_cores=8.
"""
import numpy as np

N = 32768
N1 = 128   # stage-3 DFT size
N2 = 256   # stage-1 DFT size
E = 16
B = 64
NCORES = 8
BC = B // NCORES      # 8 batches per core
K1 = 65               # k1 = 0..64 covers k = k2 + 256*k1 up to Nyquist

F32 = np.float32
F16 = np.float16
GSC = np.float32(1.0 / 16.0)   # scale folded into E1 (and 1/GSC into E2)


def _host_tables():
    n1 = np.arange(N1)
    n2 = np.arange(N2)
    k2 = np.arange(N2)
    k1 = np.arange(K1)
    W2 = np.exp(-2j * np.pi * np.outer(n2, k2) / N2)        # (n2, k2)
    W2cat = np.concatenate([W2.real, W2.imag], 1)           # (256, 512)
    T = np.exp(-2j * np.pi * np.outer(n1, k2) / N)          # (n1, k2)
    W1 = np.exp(-2j * np.pi * np.outer(n1, k1) / N1)        # (n1, k1)
    E1 = np.exp(+2j * np.pi * np.outer(np.arange(64), np.arange(N1)) / N1) * GSC
    Tinv = np.exp(+2j * np.pi * np.outer(np.arange(N1), k2) / N)            # (m, k2)
    E2 = np.exp(+2j * np.pi * np.outer(k2, np.arange(N2)) / N2) * (2.0 / N / GSC)
    return W2cat, T, W1, E1, Tinv, E2


def _build_graph():
    import concourse.bass as bass
    import concourse.mybir as mybir
    import concourse.tile as tile
    from concourse import bacc

    dt = mybir.dt
    nc = bacc.Bacc("TRN2", target_bir_lowering=False, debug=False, num_devices=NCORES)

    # ---- DRAM parameters (per-core shard shapes) ----
    # stems16[b,e] is (128, 256): partition p holds [x[p, :], x[128+p, :]]
    # where x = stem reshaped (n2=256, n1=128); cols 0:128 = n2 in 0..127.
    stems_d = nc.dram_tensor("stems16", [BC, E, N1, N2], dt.float16, kind="ExternalInput")
    c_d = nc.dram_tensor("c_tab", [BC, E, N1, 512], dt.float16, kind="ExternalInput")
    m_d = nc.dram_tensor("m_tab", [BC, E, N1, 130], dt.float16, kind="ExternalInput")
    w2_d = nc.dram_tensor("w2cat", [N2, 512], dt.float16, kind="ExternalInput")
    e1c_d = nc.dram_tensor("e1c", [64, N1], dt.float16, kind="ExternalInput")
    e1s_d = nc.dram_tensor("e1s", [64, N1], dt.float16, kind="ExternalInput")
    tic_d = nc.dram_tensor("ti_c", [N1, N2], dt.float16, kind="ExternalInput")
    tis_d = nc.dram_tensor("ti_s", [N1, N2], dt.float16, kind="ExternalInput")
    e2c_d = nc.dram_tensor("e2c", [N2, N2], dt.float16, kind="ExternalInput")
    e2sn_d = nc.dram_tensor("e2sn", [N2, N2], dt.float16, kind="ExternalInput")
    out_d = nc.dram_tensor("out", [BC, N2, N1], dt.float32, kind="ExternalOutput")
    aux_d = nc.dram_tensor("aux", [BC, 2], dt.float16, kind="ExternalOutput")

    with tile.TileContext(nc) as tc:
        with (
            tc.tile_pool(name="const", bufs=1) as cpool,
            tc.tile_pool(name="work", bufs=3) as pool,
            tc.tile_pool(name="psum", bufs=2, space="PSUM") as psum,
            tc.tile_pool(name="psacc", bufs=2, space="PSUM") as psacc,
        ):
            # ---- load constants once ----
            w2_0 = cpool.tile([128, 512], dt.float16, tag="w2_0")
            w2_1 = cpool.tile([128, 512], dt.float16, tag="w2_1")
            w2 = [w2_0, w2_1]
            nc.sync.dma_start(w2[0][:], w2_d[0:128, :])
            nc.sync.dma_start(w2[1][:], w2_d[128:256, :])
            e1c = cpool.tile([64, N1], dt.float16, tag="e1c")
            e1s = cpool.tile([64, N1], dt.float16, tag="e1s")
            nc.sync.dma_start(e1c[:], e1c_d[:])
            nc.sync.dma_start(e1s[:], e1s_d[:])
            ti_c = cpool.tile([N1, N2], dt.float16, tag="tic")
            ti_s = cpool.tile([N1, N2], dt.float16, tag="tis")
            nc.sync.dma_start(ti_c[:], tic_d[:])
            nc.sync.dma_start(ti_s[:], tis_d[:])
            e2c_0 = cpool.tile([128, N2], dt.float16, tag="e2c_0")
            e2c_1 = cpool.tile([128, N2], dt.float16, tag="e2c_1")
            e2sn_0 = cpool.tile([128, N2], dt.float16, tag="e2sn_0")
            e2sn_1 = cpool.tile([128, N2], dt.float16, tag="e2sn_1")
            e2c = [e2c_0, e2c_1]
            e2sn = [e2sn_0, e2sn_1]
            nc.sync.dma_start(e2c[0][:], e2c_d[0:128, :])
            nc.sync.dma_start(e2c[1][:], e2c_d[128:256, :])
            nc.sync.dma_start(e2sn[0][:], e2sn_d[0:128, :])
            nc.sync.dma_start(e2sn[1][:], e2sn_d[128:256, :])

            for b in range(BC):
                pA = psacc.tile([K1, 512], dt.float32, tag="pA")
                pB = psacc.tile([K1, 512], dt.float32, tag="pB")
                for e in range(E):
                    # per-event uploads (no casts: HWDGE)
                    xm = pool.tile([128, N2], dt.float16, tag="xm")
                    nc.sync.dma_start(xm[:], stems_d[b, e])
                    ce = pool.tile([N1, 512], dt.float16, tag="ce")
                    nc.sync.dma_start(ce[:], c_d[b, e])
                    mm = pool.tile([N1, 130], dt.float16, tag="mm")
                    nc.sync.dma_start(mm[:], m_d[b, e])
                    # stage 1: p1[n1, k2cat] = sum_n2 x[n2,n1] W2[n2,k2cat]
                    p1 = psum.tile([N1, 512], dt.float32, tag="p1")
                    nc.tensor.matmul(p1[:], xm[:, 0:128], w2[0][:], start=True, stop=False)
                    nc.tensor.matmul(p1[:], xm[:, 128:256], w2[1][:], start=False, stop=True)
                    p1sb = pool.tile([N1, 512], dt.float16, tag="p1sb")
                    nc.any.tensor_copy(p1sb[:], p1[:])
                    # UV = p1 * C  (complex; C = T*A_e uploaded)
                    uv = pool.tile([N1, 512], dt.float16, tag="uv")
                    tmp1 = pool.tile([N1, N2], dt.float16, tag="tmp1")
                    tmp2 = pool.tile([N1, N2], dt.float16, tag="tmp2")
                    nc.any.tensor_mul(tmp1[:], p1sb[:, 0:256], ce[:, 0:256])
                    nc.any.tensor_mul(tmp2[:], p1sb[:, 256:512], ce[:, 256:512])
                    nc.any.tensor_sub(uv[:, 0:256], tmp1[:], tmp2[:])
                    nc.any.tensor_mul(tmp1[:], p1sb[:, 0:256], ce[:, 256:512])
                    nc.any.tensor_mul(tmp2[:], p1sb[:, 256:512], ce[:, 0:256])
                    nc.any.tensor_add(uv[:, 256:512], tmp1[:], tmp2[:])
                    # stage 3 (M = W1*B_e uploaded), accumulating over events
                    nc.tensor.matmul(pA[:], mm[:, 0:65], uv[:],
                                     start=(e == 0), stop=(e == E - 1))
                    nc.tensor.matmul(pB[:], mm[:, 65:130], uv[:],
                                     start=(e == 0), stop=(e == E - 1))
                # combine into X (65, 512): [Xre|Xim]
                xf = pool.tile([K1, 512], dt.float16, tag="xf")
                pbsb = pool.tile([K1, 512], dt.float16, tag="pbsb")
                nc.any.tensor_copy(pbsb[:], pB[:])
                nc.any.tensor_sub(xf[:, 0:256], pA[:, 0:256], pbsb[:, 256:512])
                nc.any.tensor_add(xf[:, 256:512], pA[:, 256:512], pbsb[:, 0:256])
                nc.sync.dma_start(aux_d[b, 0:1], xf[0:1, 0:1])
                nc.sync.dma_start(aux_d[b, 1:2], xf[64:65, 0:1])
                # inverse stage I1: G = E1^T @ X   (contract k1=64)
                pga = psum.tile([N1, 512], dt.float32, tag="pinv")
                pgb = psum.tile([N1, 512], dt.float32, tag="pinv")
                nc.tensor.matmul(pga[:], e1c[:], xf[0:64, :], start=True, stop=True)
                nc.tensor.matmul(pgb[:], e1s[:], xf[0:64, :], start=True, stop=True)
                g_re = pool.tile([N1, N2], dt.float16, tag="gre")
                g_im = pool.tile([N1, N2], dt.float16, tag="gim")
                gbsb = pool.tile([N1, 512], dt.float16, tag="gbsb")
                nc.any.tensor_copy(gbsb[:], pgb[:])
                nc.any.tensor_sub(g_re[:], pga[:, 0:256], gbsb[:, 256:512])
                nc.any.tensor_add(g_im[:], pga[:, 256:512], gbsb[:, 0:256])
                # twiddle: GT = G * Tinv
                gt_re = pool.tile([N1, N2], dt.float16, tag="gtre")
                gt_im = pool.tile([N1, N2], dt.float16, tag="gtim")
                it1 = pool.tile([N1, N2], dt.float16, tag="it1")
                it2 = pool.tile([N1, N2], dt.float16, tag="it2")
                nc.any.tensor_mul(it1[:], g_re[:], ti_c[:])
                nc.any.tensor_mul(it2[:], g_im[:], ti_s[:])
                nc.any.tensor_sub(gt_re[:], it1[:], it2[:])
                nc.any.tensor_mul(it1[:], g_re[:], ti_s[:])
                nc.any.tensor_mul(it2[:], g_im[:], ti_c[:])
                nc.any.tensor_add(gt_im[:], it1[:], it2[:])
                # transpose GT -> (k2, m), 2 chunks each
                gtt_re_0 = pool.tile([128, N1], dt.float16, tag="gttre0")
                gtt_re_1 = pool.tile([128, N1], dt.float16, tag="gttre1")
                gtt_im_0 = pool.tile([128, N1], dt.float16, tag="gttim0")
                gtt_im_1 = pool.tile([128, N1], dt.float16, tag="gttim1")
                gtt_re = [gtt_re_0, gtt_re_1]
                gtt_im = [gtt_im_0, gtt_im_1]
                for kc in range(2):
                    nc.sync.dma_start_transpose(
                        gtt_re[kc][:], gt_re[:, 128 * kc : 128 * kc + 128])
                    nc.sync.dma_start_transpose(
                        gtt_im[kc][:], gt_im[:, 128 * kc : 128 * kc + 128])
                # I4: S[j, m] = sum_k2 E2[k2,j] GTt[k2,m]  (real part only)
                for jc in range(2):
                    ps = psum.tile([128, 512], dt.float32, tag="pinv")
                    for kc in range(2):
                        nc.tensor.matmul(
                            ps[:, 0:128], e2c[kc][:, 128 * jc : 128 * jc + 128],
                            gtt_re[kc][:],
                            start=(kc == 0), stop=False)
                        nc.tensor.matmul(
                            ps[:, 0:128], e2sn[kc][:, 128 * jc : 128 * jc + 128],
                            gtt_im[kc][:],
                            start=False, stop=(kc == 1))
                    y_sb = pool.tile([128, N1], dt.float32, tag="ysb")
                    nc.any.tensor_copy(y_sb[:], ps[:, 0:128])
                    nc.sync.dma_start(out_d[b, 128 * jc : 128 * jc + 128, :], y_sb[:])
    nc.compile()
    return nc


def kernel(time_latent, stems, targets, W_pos, b_pos):
    from concourse.bass_utils import run_bass_kernel_spmd

    # host: positions (tiny linear+sigmoid, fp32 exactly like the reference)
    z = np.einsum("bed,od->beo", time_latent.astype(F32), W_pos.astype(F32))
    z = z.reshape(B, E) + b_pos.reshape(1)[0]
    pos = 1.0 / (1.0 + np.exp(-z, dtype=F32))
    s = pos * np.float32(N)

    W2cat, T, W1, E1, Tinv, E2 = _host_tables()
    k2 = np.arange(N2)
    k1 = np.arange(K1)

    # stems: (B,E,32768) f32 -> (B,E,128,256) fp16 with cols [n2<128 | n2>=128]
    x = stems.reshape(B, E, N2, N1).astype(F16)          # x[.., n2, n1]
    x = x.reshape(B, E, 2, 128, N1).transpose(0, 1, 3, 2, 4).reshape(B, E, N1, N2)

    nc = _build_graph()
    in_maps = []
    for c in range(NCORES):
        sl = slice(c * BC, (c + 1) * BC)
        s_c = s[sl].astype(np.float64)                    # (BC, E)
        A = np.exp(-2j * np.pi * s_c[..., None] * k2 / N)     # (BC,E,256)
        Bt = np.exp(-2j * np.pi * s_c[..., None] * k1 / N1)   # (BC,E,65)
        C = T[None, None] * A[:, :, None, :]                  # (BC,E,128,256)
        M = W1[None, None] * Bt[:, :, None, :]                # (BC,E,128,65)
        in_maps.append({
            "stems16": np.ascontiguousarray(x[sl]),
            "c_tab": np.concatenate([C.real, C.imag], -1).astype(F16),
            "m_tab": np.concatenate([M.real, M.imag], -1).astype(F16),
            "w2cat": W2cat.astype(F16),
            "e1c": E1.real.astype(F16),
            "e1s": E1.imag.astype(F16),
            "ti_c": Tinv.real.astype(F16),
            "ti_s": Tinv.imag.astype(F16),
            "e2c": E2.real.astype(F16),
            "e2sn": (-E2.imag).astype(F16),
        })

    import os
    trace = bool(int(os.environ.get("ATHENA_TRACE", "0")))
    res = run_bass_kernel_spmd(nc, in_maps, core_ids=list(range(NCORES)), trace=trace)
    if trace:
        print(f"HW exec time: {res.exec_time_ns} ns")
    outs = []
    sign = np.where(np.arange(N) % 2 == 0, 1.0, -1.0).astype(F32)
    for c in range(NCORES):
        y = res.results[c]["out"].reshape(BC, N).astype(F32)
        aux = res.results[c]["aux"].astype(F32)          # (BC, 2) = X0, XN2re
        y = y + (-aux[:, 0:1] + sign[None, :] * aux[:, 1:2]) / np.float32(N)
        outs.append(y)
    return np.concatenate(outs, 0).reshape(B, 1, N).astype(F32)


# revision 5
# speedup vs baseline: 2.0379x; 1.4972x over previous
"""AtomPlacementScheduler Trainium2 kernel (v3).

out[b] = sum_e irfft(rfft(stems[b,e]) * exp(-2i pi f s_be)),  s = sigmoid(TL@W+b)*N.

4-step FFT, half-spectrum form: the full signed-frequency grid
k~ = k2 + 256*k1 with k2 in [0,128] (129 cols, padded to 132) and SIGNED
k1 in [-64,63] (128 rows) covers every conjugate pair of the real-signal
spectrum exactly once (k2 in {0,128} columns are self-paired, weight 1;
k2 in [1,127] carry weight 2 + real part).  Shift phase factors exactly as
A[k2]*B[k1] on this grid (no partial-row corrections), so per event the
device does: 2 stage-1 matmuls (264 free), 1 PSUM->SBUF copy, 6 half-width
elementwise ops (DVE re-chain, GpSimd im-chain), 2 stage-3 matmuls (264
free) accumulating the event sum in PSUM.  All twiddle tables (C = T*A_e,
M = W1*B_e) are host-precomputed and DMA'd fp16 in one fused transfer.
The inverse (per batch) is I1 -> twiddle*d/N -> transpose -> I4, exact
(no host correction).

Self-contained: hardcodes shapes B=64, E=16, N=32768, n_cores=8.
"""
import numpy as np

N = 32768
N1 = 128
N2 = 256
E = 16
B = 64
NCORES = 8
BC = B // NCORES
K2 = 129            # k2 = 0..128
KP = 132            # padded k2 width
F32 = np.float32
F16 = np.float16
GSC = np.float32(1.0 / 16.0)


def _host_tables():
    n1 = np.arange(N1)
    n2 = np.arange(N2)
    k2 = np.arange(K2)
    kap = np.arange(N1) - 64                       # signed k1
    W2 = np.exp(-2j * np.pi * np.outer(n2, k2) / N2)        # (256, 129)
    T = np.exp(-2j * np.pi * np.outer(n1, k2) / N)          # (128, 129)
    W1s = np.exp(-2j * np.pi * np.outer(n1, kap) / N1)      # (128, 128)
    E1s = np.exp(+2j * np.pi * np.outer(kap, n1) / N1) * GSC  # (128, 128) [j, n1]
    d = np.where((k2 == 0) | (k2 == 128), 1.0, 2.0)
    TW = np.exp(+2j * np.pi * np.outer(n1, k2) / N) * (d / (N * GSC))  # (128,129)
    E2 = np.exp(+2j * np.pi * np.outer(np.arange(K2), n2) / N2)        # (129, 256)
    return W2, T, W1s, E1s, TW, E2


def _pad(a, w=KP):
    # pad last axis to w with zeros
    out = np.zeros(a.shape[:-1] + (w,), dtype=a.dtype)
    out[..., : a.shape[-1]] = a
    return out


def _build_graph():
    import concourse.bass as bass
    import concourse.mybir as mybir
    import concourse.tile as tile
    from concourse import bacc

    dt = mybir.dt
    nc = bacc.Bacc("TRN2", target_bir_lowering=False, debug=False, num_devices=NCORES)

    W = 2 * KP          # 264: [re | im]
    stems_d = nc.dram_tensor("stems16", [BC, E, N1, N2], dt.float16, kind="ExternalInput")
    cm_d = nc.dram_tensor("cm_tab", [BC, E, N1, W + 256], dt.float16, kind="ExternalInput")
    w2_d = nc.dram_tensor("w2cat", [N2, W], dt.float16, kind="ExternalInput")
    e1c_d = nc.dram_tensor("e1sc", [N1, N1], dt.float16, kind="ExternalInput")
    e1s_d = nc.dram_tensor("e1ss", [N1, N1], dt.float16, kind="ExternalInput")
    twc_d = nc.dram_tensor("twc", [N1, KP], dt.float16, kind="ExternalInput")
    tws_d = nc.dram_tensor("tws", [N1, KP], dt.float16, kind="ExternalInput")
    e2c0_d = nc.dram_tensor("e2c0", [128, N2], dt.float16, kind="ExternalInput")
    e2sn0_d = nc.dram_tensor("e2sn0", [128, N2], dt.float16, kind="ExternalInput")
    e2c1_d = nc.dram_tensor("e2c1", [128, N2], dt.float16, kind="ExternalInput")
    out_d = nc.dram_tensor("out", [BC, N2, N1], dt.float32, kind="ExternalOutput")

    with tile.TileContext(nc) as tc:
        with (
            tc.tile_pool(name="const", bufs=1) as cpool,
            tc.tile_pool(name="work", bufs=4) as pool,
            tc.tile_pool(name="binv", bufs=2) as bpool,
            tc.tile_pool(name="psum", bufs=3, space="PSUM") as psum,
            tc.tile_pool(name="psacc", bufs=1, space="PSUM") as psacc,
            tc.tile_pool(name="pinv", bufs=1, space="PSUM") as pinv,
        ):
            w2h0 = cpool.tile([128, W], dt.float16, tag="w2h0")
            w2h1 = cpool.tile([128, W], dt.float16, tag="w2h1")
            nc.sync.dma_start(w2h0[:], w2_d[0:128, :])
            nc.sync.dma_start(w2h1[:], w2_d[128:256, :])
            e1sc = cpool.tile([N1, N1], dt.float16, tag="e1sc")
            e1ss = cpool.tile([N1, N1], dt.float16, tag="e1ss")
            nc.sync.dma_start(e1sc[:], e1c_d[:])
            nc.sync.dma_start(e1ss[:], e1s_d[:])
            twc = cpool.tile([N1, KP], dt.float16, tag="twc")
            tws = cpool.tile([N1, KP], dt.float16, tag="tws")
            nc.sync.dma_start(twc[:], twc_d[:])
            nc.sync.dma_start(tws[:], tws_d[:])
            e2c0 = cpool.tile([128, N2], dt.float16, tag="e2c0")
            e2sn0 = cpool.tile([128, N2], dt.float16, tag="e2sn0")
            e2c1 = cpool.tile([128, N2], dt.float16, tag="e2c1")
            nc.sync.dma_start(e2c0[:], e2c0_d[:])
            nc.sync.dma_start(e2sn0[:], e2sn0_d[:])
            nc.sync.dma_start(e2c1[:], e2c1_d[:])

            for b in range(BC):
                pZA = psacc.tile([N1, W], dt.float32, tag="pZA")
                pZB = psacc.tile([N1, W], dt.float32, tag="pZB")
                for e in range(E):
                    xm = pool.tile([128, N2], dt.float16, tag="xm")
                    nc.scalar.dma_start(xm[:], stems_d[b, e])
                    cm = pool.tile([N1, W + 256], dt.float16, tag="cm")
                    nc.sync.dma_start(cm[:], cm_d[b, e])
                    p1 = psum.tile([N1, W], dt.float32, tag="p1")
                    nc.tensor.matmul(p1[:], xm[:, 0:128], w2h0[:], start=True, stop=False)
                    nc.tensor.matmul(p1[:], xm[:, 128:256], w2h1[:], start=False, stop=True)
                    p1sb = pool.tile([N1, W], dt.float16, tag="p1sb")
                    nc.any.tensor_copy(p1sb[:], p1[:])
                    # U = P1 * C  (C = cm[:, 0:264]); re on DVE, im on GpSimd
                    uv = pool.tile([N1, W], dt.float16, tag="uv")
                    t1 = pool.tile([N1, KP], dt.float16, tag="t1")
                    t2 = pool.tile([N1, KP], dt.float16, tag="t2")
                    t3 = pool.tile([N1, KP], dt.float16, tag="t3")
                    t4 = pool.tile([N1, KP], dt.float16, tag="t4")
                    nc.vector.tensor_mul(t1[:], p1sb[:, 0:KP], cm[:, 0:KP])
                    nc.vector.tensor_mul(t2[:], p1sb[:, KP:W], cm[:, KP:W])
                    nc.vector.tensor_sub(uv[:, 0:KP], t1[:], t2[:])
                    nc.gpsimd.tensor_mul(t3[:], p1sb[:, 0:KP], cm[:, KP:W])
                    nc.gpsimd.tensor_mul(t4[:], p1sb[:, KP:W], cm[:, 0:KP])
                    nc.gpsimd.tensor_add(uv[:, KP:W], t3[:], t4[:])
                    # stage 3: accumulate over events; M_re/M_im from cm tail
                    nc.tensor.matmul(pZA[:], cm[:, W : W + 128], uv[:],
                                     start=(e == 0), stop=(e == E - 1))
                    nc.tensor.matmul(pZB[:], cm[:, W + 128 : W + 256], uv[:],
                                     start=(e == 0), stop=(e == E - 1))
                # xf = Z (128, 264)
                xf = bpool.tile([N1, W], dt.float16, tag="xf")
                pbsb = bpool.tile([N1, W], dt.float16, tag="pbsb")
                nc.any.tensor_copy(pbsb[:], pZB[:])
                nc.any.tensor_sub(xf[:, 0:KP], pZA[:, 0:KP], pbsb[:, KP:W])
                nc.any.tensor_add(xf[:, KP:W], pZA[:, KP:W], pbsb[:, 0:KP])
                # I1: G = E1s^T @ Z
                pga = pinv.tile([N1, W], dt.float32, tag="pga")
                pgb = pinv.tile([N1, W], dt.float32, tag="pgb")
                nc.tensor.matmul(pga[:], e1sc[:], xf[:], start=True, stop=True)
                nc.tensor.matmul(pgb[:], e1ss[:], xf[:], start=True, stop=True)
                g_re = bpool.tile([N1, KP], dt.float16, tag="gre")
                g_im = bpool.tile([N1, KP], dt.float16, tag="gim")
                gbsb = bpool.tile([N1, W], dt.float16, tag="gbsb")
                nc.any.tensor_copy(gbsb[:], pgb[:])
                nc.any.tensor_sub(g_re[:], pga[:, 0:KP], gbsb[:, KP:W])
                nc.any.tensor_add(g_im[:], pga[:, KP:W], gbsb[:, 0:KP])
                # GT = G * TW  (d/N folded in); gt_re padded to 256 for transpose
                gt_re = bpool.tile([N1, N2], dt.float16, tag="gtre")
                gt_im = bpool.tile([N1, KP], dt.float16, tag="gtim")
                i1 = bpool.tile([N1, KP], dt.float16, tag="i1")
                i2 = bpool.tile([N1, KP], dt.float16, tag="i2")
                nc.vector.tensor_mul(i1[:], g_re[:], twc[:])
                nc.vector.tensor_mul(i2[:], g_im[:], tws[:])
                nc.vector.tensor_sub(gt_re[:, 0:KP], i1[:], i2[:])
                nc.gpsimd.tensor_mul(i1[:], g_re[:], tws[:])
                nc.gpsimd.tensor_mul(i2[:], g_im[:], twc[:])
                nc.gpsimd.tensor_add(gt_im[:], i1[:], i2[:])
                # transposes: (k2, n1) chunks
                gttre0 = bpool.tile([128, N1], dt.float16, tag="gttre0")
                gttre1 = bpool.tile([128, N1], dt.float16, tag="gttre1")
                gttim0 = bpool.tile([128, N1], dt.float16, tag="gttim0")
                nc.sync.dma_start_transpose(gttre0[:], gt_re[:, 0:128])
                nc.sync.dma_start_transpose(gttre1[:], gt_re[:, 128:256])
                nc.sync.dma_start_transpose(gttim0[:], gt_im[:, 0:128])
                # I4: y[n2, n1] = sum_k2 Re(E2 * GT^T)
                for jc in range(2):
                    js = slice(128 * jc, 128 * jc + 128)
                    ps = pinv.tile([128, N1], dt.float32, tag="ps")
                    nc.tensor.matmul(ps[:], e2c0[:, js], gttre0[:], start=True, stop=False)
                    nc.tensor.matmul(ps[:], e2sn0[:, js], gttim0[:], start=False, stop=False)
                    nc.tensor.matmul(ps[:], e2c1[:, js], gttre1[:], start=False, stop=True)
                    y_sb = bpool.tile([128, N1], dt.float32, tag="ysb")
                    nc.any.tensor_copy(y_sb[:], ps[:])
                    nc.sync.dma_start(out_d[b, js, :], y_sb[:])
    nc.compile()
    return nc


def kernel(time_latent, stems, targets, W_pos, b_pos):
    from concourse.bass_utils import run_bass_kernel_spmd

    z = np.einsum("bed,od->beo", time_latent.astype(F32), W_pos.astype(F32))
    z = z.reshape(B, E) + b_pos.reshape(1)[0]
    pos = 1.0 / (1.0 + np.exp(-z, dtype=F32))
    s = pos * np.float32(N)

    W2, T, W1s, E1s, TW, E2 = _host_tables()
    k2 = np.arange(K2)
    kap = np.arange(N1) - 64

    # stems: (B,E,32768) -> (B,E,128,256) fp16, cols [n2<128 | n2>=128]
    x = stems.reshape(B, E, N2, N1).astype(F16)
    x = x.reshape(B, E, 2, 128, N1).transpose(0, 1, 3, 2, 4).reshape(B, E, N1, N2)

    w2cat = np.concatenate([_pad(W2.real), _pad(W2.imag)], 1)  # (256, 264)

    nc = _build_graph()
    in_maps = []
    for c in range(NCORES):
        sl = slice(c * BC, (c + 1) * BC)
        s_c = s[sl].astype(np.float64)                        # (BC, E)
        A = np.exp(-2j * np.pi * s_c[..., None] * k2 / N)     # (BC,E,129)
        Bs = np.exp(-2j * np.pi * s_c[..., None] * kap / N1)  # (BC,E,128)
        C = T[None, None] * A[:, :, None, :]                  # (BC,E,128,129)
        M = W1s[None, None] * Bs[:, :, None, :]               # (BC,E,128,128)
        cm = np.concatenate(
            [_pad(C.real), _pad(C.imag), M.real, M.imag], -1).astype(F16)
        in_maps.append({
            "stems16": np.ascontiguousarray(x[sl]),
            "cm_tab": cm,                                     # (BC,E,128,520)
            "w2cat": w2cat.astype(F16),
            "e1sc": E1s.real.astype(F16),
            "e1ss": E1s.imag.astype(F16),
            "twc": _pad(TW.real).astype(F16),
            "tws": _pad(TW.imag).astype(F16),
            "e2c0": E2.real[0:128].astype(F16),
            "e2sn0": (-E2.imag[0:128]).astype(F16),
            "e2c1": np.concatenate([E2.real[128:129], np.zeros((127, N2))], 0).astype(F16),
        })

    import os
    trace = bool(int(os.environ.get("ATHENA_TRACE", "0")))
    res = run_bass_kernel_spmd(nc, in_maps, core_ids=list(range(NCORES)), trace=trace)
    if trace:
        print(f"HW exec time: {res.exec_time_ns} ns")
    outs = [res.results[c]["out"].reshape(BC, N).astype(F32) for c in range(NCORES)]
    return np.concatenate(outs, 0).reshape(B, 1, N).astype(F32)
